# revision 17
# baseline (speedup 1.0000x reference)
"""KNN-attention Trainium2 kernel.

Sharding: 8 cores = 2 batches x 4 head-pairs. Each core computes, for its
batch b and its 2 heads: q/k/v projections, causal local attention with T5
relative bias, kNN memory attention (null key prepended), sigmoid-gated
combination, and its partial output projection (its 2 heads' rows of Wout).
Host sums the 4 partials per batch and adds bout.

Device layout: attention runs transposed (simT[j,i] = k_j . q_i) so the
softmax denominator falls out of the attn@V matmul via a ones-column on V,
and the relative-position bias + causal mask are a single Toeplitz slab
added pre-exp (logits are small enough that max-subtraction is unneeded;
masked entries use -1e33 so exp underflows to exactly 0).
"""

import sys
from contextlib import ExitStack

import numpy as np

sys.path.insert(0, "/opt/trn_rl_repo")

import concourse.bass as bass  # noqa: E402
import concourse.bacc as bacc  # noqa: E402
import concourse.mybir as mybir  # noqa: E402
import concourse.tile as tile  # noqa: E402
from concourse import bass_utils  # noqa: E402
from concourse.masks import make_identity  # noqa: E402

B, N, DIM = 2, 2048, 1024
H, DH = 8, 64
NUM_BUCKETS, MAX_DISTANCE = 32, 128
HPC = 2  # heads per core
SCALE = DH ** -0.5
MKB = 17  # mem key blocks (null + 2048 keys + pad -> 2176)
MK = MKB * 128
SLABW = 2432  # toeplitz slab width: d in [-384, 2047] -> u = d + 384
NEG = -1.0e33
F32 = mybir.dt.float32
FP = mybir.ActivationFunctionType


def _bucket_table():
    """table[d] = bucket index for key-query distance d >= 0.

    Computed with jnp so per-element numerics (log approximation, int cast)
    match the reference run in the same environment exactly — the bucket of
    boundary distances depends on the backend's log accuracy.
    """
    max_exact = NUM_BUCKETS // 2
    try:
        import jax.numpy as jnp

        n = jnp.arange(N)
        is_small = n < max_exact
        n_safe = jnp.maximum(n, 1)
        val = max_exact + (
            jnp.log(n_safe.astype(jnp.float32) / max_exact)
            / np.log(MAX_DISTANCE / max_exact)
            * (NUM_BUCKETS - max_exact)
        ).astype(jnp.int32)
        val = jnp.minimum(val, NUM_BUCKETS - 1)
        return np.asarray(jnp.where(is_small, n, val)).astype(np.int32)
    except Exception:
        d = np.arange(N)
        n_safe = np.maximum(d, 1).astype(np.float32)
        val = max_exact + (
            np.log(n_safe / np.float32(max_exact))
            / np.float32(np.log(MAX_DISTANCE / max_exact))
            * np.float32(NUM_BUCKETS - max_exact)
        ).astype(np.int32)
        val = np.minimum(val, NUM_BUCKETS - 1)
        return np.where(d < max_exact, d, val).astype(np.int32)


def build_program():
    nc = bacc.Bacc("TRN2", target_bir_lowering=False, debug=False)

    xb_d = nc.dram_tensor("xb", [N, DIM], F32, kind="ExternalInput")
    wq_d = nc.dram_tensor("wq", [DIM, HPC * DH], F32, kind="ExternalInput")
    wkv_d = nc.dram_tensor("wkv", [DIM, 2 * DH], F32, kind="ExternalInput")
    wout_d = nc.dram_tensor("wout", [HPC * DH, DIM], F32, kind="ExternalInput")
    memkt_d = nc.dram_tensor("memkt", [HPC, DH, MK], F32, kind="ExternalInput")
    memv_d = nc.dram_tensor("memv", [HPC, MK, DH + 1], F32, kind="ExternalInput")
    mbias_d = nc.dram_tensor("mbias", [HPC, 128, MKB], F32, kind="ExternalInput")
    slab_d = nc.dram_tensor("slab", [HPC, 128, SLABW], F32, kind="ExternalInput")
    gvec_d = nc.dram_tensor("gvec", [1, HPC * 128], F32, kind="ExternalInput")
    out_d = nc.dram_tensor("out", [N, DIM], F32, kind="ExternalOutput")

    with tile.TileContext(nc) as tc, ExitStack() as ctx:
        const = ctx.enter_context(tc.tile_pool(name="const", bufs=1))
        pers = ctx.enter_context(tc.tile_pool(name="pers", bufs=1))
        psum_sim = ctx.enter_context(tc.tile_pool(name="psum_sim", bufs=2, space="PSUM"))
        psum_av = ctx.enter_context(tc.tile_pool(name="psum_av", bufs=1, space="PSUM"))
        psum_misc = ctx.enter_context(tc.tile_pool(name="psum_misc", bufs=2, space="PSUM"))

        ident = const.tile([128, 128], F32)
        make_identity(nc, ident)
        zbias = const.tile([128, 1], F32)
        nc.vector.memset(zbias[:], 0.0)

        # --- persistent SBUF tensors ---
        wq_sb = pers.tile([128, 8, HPC * DH], F32)
        nc.sync.dma_start(wq_sb[:], wq_d[:].rearrange("(a p) m -> p a m", p=128))
        wkv_sb = pers.tile([128, 8, 2 * DH], F32)
        nc.sync.dma_start(wkv_sb[:], wkv_d[:].rearrange("(a p) m -> p a m", p=128))
        wout_sb = pers.tile([128, DIM], F32)
        nc.sync.dma_start(wout_sb[:], wout_d[:])
        slab_sb = pers.tile([128, HPC, SLABW], F32)
        nc.sync.dma_start(slab_sb[:], slab_d[:].rearrange("h p u -> p h u"))
        memkt_sb = pers.tile([DH, HPC, MK], F32)
        nc.sync.dma_start(memkt_sb[:], memkt_d[:].rearrange("h d k -> d h k"))
        memv_sb = pers.tile([128, HPC, MKB, DH + 1], F32)
        nc.sync.dma_start(
            memv_sb[:], memv_d[:].rearrange("h (a p) c -> p h a c", p=128)
        )
        mbias_sb = pers.tile([128, HPC, MKB], F32)
        nc.sync.dma_start(mbias_sb[:], mbias_d[:].rearrange("h p k -> p h k"))
        gvec_sb = pers.tile([DH + 1, HPC * 128], F32)  # data on partition 64
        nc.sync.dma_start(gvec_sb[DH : DH + 1, :], gvec_d[:])

        qTh = pers.tile([DH, HPC, N], F32)  # per-head qT, base partition 0
        kT = pers.tile([DH, N], F32)
        vloc = pers.tile([128, 16, DH + 1], F32)  # [key_part, key_block, DH+ones]
        nc.vector.memset(vloc[:, :, DH], 1.0)

        with (
            tc.tile_pool(name="xtpool", bufs=1) as xtpool,
            tc.tile_pool(name="xstage", bufs=3) as xstage,
            tc.tile_pool(name="qvstage", bufs=2) as qvstage,
        ):
            xT = xtpool.tile([128, 8, N], F32)  # [f_part, f_block, seq]
            vT = xtpool.tile([DH, N], F32)
            # --- transpose x into xT ---
            for t in range(16):
                xrow = xstage.tile([128, DIM], F32, tag="xrow")
                nc.sync.dma_start(xrow[:], xb_d[t * 128 : (t + 1) * 128, :])
                for fb in range(8):
                    ptr = psum_misc.tile([128, 128], F32, tag="misc")
                    nc.tensor.transpose(
                        ptr[:], xrow[:, fb * 128 : (fb + 1) * 128], ident[:]
                    )
                    nc.any.tensor_copy(
                        out=xT[:, fb, t * 128 : (t + 1) * 128], in_=ptr[:]
                    )

            # --- projections: qT (2 heads), kT/vT (shared) ---
            # Heads/v land on psum partitions 64-127; DMA re-bases them to 0.
            for icq in range(4):
                sl = slice(icq * 512, (icq + 1) * 512)
                pq = psum_misc.tile([128, 512], F32, tag="misc")
                for fb in range(8):
                    nc.tensor.matmul(
                        pq[:], wq_sb[:, fb, :], xT[:, fb, sl],
                        start=(fb == 0), stop=(fb == 7),
                    )
                nc.any.tensor_copy(out=qTh[:, 0, sl], in_=pq[:DH, :])
                qstage = qvstage.tile([128, 512], F32, tag="qstage")
                nc.any.tensor_copy(out=qstage[DH:, :], in_=pq[DH:, :])
                nc.sync.dma_start(qTh[:, 1, sl], qstage[DH:, :])
                pkv = psum_misc.tile([128, 512], F32, tag="misc")
                for fb in range(8):
                    nc.tensor.matmul(
                        pkv[:], wkv_sb[:, fb, :], xT[:, fb, sl],
                        start=(fb == 0), stop=(fb == 7),
                    )
                nc.any.tensor_copy(out=kT[:, sl], in_=pkv[:DH, :])
                vstage = qvstage.tile([128, 512], F32, tag="vstage")
                nc.any.tensor_copy(out=vstage[DH:, :], in_=pkv[DH:, :])
                nc.sync.dma_start(vT[:, sl], vstage[DH:, :])

            # vloc[p, t, :DH] = v[t*128 + p, :] via PE transpose of vT
            for t in range(16):
                ptr = psum_misc.tile([128, DH], F32, tag="misc")
                nc.tensor.transpose(
                    ptr[:], vT[:, t * 128 : (t + 1) * 128], ident[:DH, :DH]
                )
                nc.any.tensor_copy(out=vloc[:, t, :DH], in_=ptr[:])

        # --- main attention loop (pools opened after prologue pools freed) ---
        expool = ctx.enter_context(tc.tile_pool(name="expool", bufs=4))
        combpool = ctx.enter_context(tc.tile_pool(name="combpool", bufs=2))
        opool = ctx.enter_context(tc.tile_pool(name="opool", bufs=2))
        smallpool = ctx.enter_context(tc.tile_pool(name="smallpool", bufs=1))
        for ic in range(4):
            qsl = slice(ic * 512, (ic + 1) * 512)
            comb = combpool.tile([128, 512], F32, tag="comb")
            for h in range(HPC):
                hq = qTh[:, h, qsl]
                # local causal branch
                pl = psum_av.tile([DH + 1, 512], F32, tag="avL")
                nkb = 4 * (ic + 1)
                for kb in range(nkb):
                    ps = psum_sim.tile([128, 512], F32, tag="ps")
                    nc.tensor.matmul(
                        ps[:], kT[:, kb * 128 : (kb + 1) * 128], hq,
                        start=True, stop=True,
                    )
                    off = ic * 512 - kb * 128 + 384
                    nc.vector.tensor_add(ps[:], ps[:], slab_sb[:, h, off : off + 512])
                    ex = expool.tile([128, 512], F32, tag="ex")
                    nc.scalar.activation(ex[:], ps[:], FP.Exp, bias=zbias[:], scale=SCALE)
                    nc.tensor.matmul(
                        pl[:], vloc[:, kb, :], ex[:],
                        start=(kb == 0), stop=(kb == nkb - 1),
                    )
                # memory branch
                pm = psum_av.tile([DH + 1, 512], F32, tag="avM")
                for kb in range(MKB):
                    ps = psum_sim.tile([128, 512], F32, tag="ps")
                    nc.tensor.matmul(
                        ps[:], memkt_sb[:, h, kb * 128 : (kb + 1) * 128], hq,
                        start=True, stop=True,
                    )
                    ex = expool.tile([128, 512], F32, tag="ex")
                    nc.scalar.activation(
                        ex[:], ps[:], FP.Exp,
                        bias=mbias_sb[:, h, kb : kb + 1], scale=SCALE,
                    )
                    nc.tensor.matmul(
                        pm[:], memv_sb[:, h, kb, :], ex[:],
                        start=(kb == 0), stop=(kb == MKB - 1),
                    )
                # combine: comb_h = g*pl/denl + (1-g)*pm/denm
                # denominators sit on psum partition 64; stay there for the
                # reciprocal + K=1 broadcast matmuls (base 64 is legal)
                den = smallpool.tile([DH + 1, 1024], F32, tag="den")
                nc.any.tensor_copy(out=den[DH:, 0:512], in_=pl[DH:, :])
                nc.any.tensor_copy(out=den[DH:, 512:1024], in_=pm[DH:, :])
                rr = smallpool.tile([DH + 1, 1024], F32, tag="rr")
                nc.vector.reciprocal(rr[DH:, :], den[DH:, :])
                bcpL = psum_misc.tile([DH, 512], F32, tag="misc")
                nc.tensor.matmul(
                    bcpL[:], gvec_sb[DH:, h * 128 : h * 128 + DH], rr[DH:, 0:512],
                    start=True, stop=True,
                )
                bcpM = psum_misc.tile([DH, 512], F32, tag="misc")
                nc.tensor.matmul(
                    bcpM[:], gvec_sb[DH:, h * 128 + DH : (h + 1) * 128], rr[DH:, 512:1024],
                    start=True, stop=True,
                )
                bcsL = smallpool.tile([DH, 512], F32, tag="bcsL")
                nc.any.tensor_copy(out=bcsL[:], in_=bcpL[:])
                bcsM = smallpool.tile([DH, 512], F32, tag="bcsM")
                nc.any.tensor_copy(out=bcsM[:], in_=bcpM[:])
                t2 = smallpool.tile([DH, 512], F32, tag="t2")
                if h == 0:
                    ch = comb[:DH, :]
                else:
                    ch = smallpool.tile([DH, 512], F32, tag="combh1")
                nc.vector.tensor_mul(ch[:, :], pl[:DH, :], bcsL[:])
                nc.vector.tensor_mul(t2[:], pm[:DH, :], bcsM[:])
                nc.vector.tensor_add(ch[:, :], ch[:, :], t2[:])
                if h == 1:
                    nc.sync.dma_start(comb[DH:, :], ch[:, :])
            # output projection for this row chunk
            for rb in range(4):
                osb = opool.tile([128, DIM], F32, tag="osb")
                for half in range(2):
                    po = psum_misc.tile([128, 512], F32, tag="misc")
                    nc.tensor.matmul(
                        po[:], comb[:, rb * 128 : (rb + 1) * 128],
                        wout_sb[:, half * 512 : (half + 1) * 512],
                        start=True, stop=True,
                    )
                    nc.any.tensor_copy(out=osb[:, half * 512 : (half + 1) * 512], in_=po[:])
                r0 = ic * 512 + rb * 128
                nc.sync.dma_start(out_d[r0 : r0 + 128, :], osb[:])

    nc.compile()
    return nc


def make_in_maps(x, mem_kv, mem_mask, Wq, Wkv, Wout, rel_emb, gate, null_k, null_v):
    """Host-side shard + layout prep. Returns 8 per-core input dicts."""
    x = np.asarray(x, np.float32)
    mem_kv = np.asarray(mem_kv, np.float32)
    mem_mask = np.asarray(mem_mask, bool)
    Wq, Wkv, Wout = (np.asarray(a, np.float32) for a in (Wq, Wkv, Wout))
    rel_emb = np.asarray(rel_emb, np.float32)
    gate = np.asarray(gate, np.float32)
    null_k = np.asarray(null_k, np.float32)
    null_v = np.asarray(null_v, np.float32)

    table = _bucket_table()  # [N]
    # slab[h, p, u] = 64 * rel_emb[table[u-384-p], h] if d>=0 else NEG
    d = np.arange(SLABW)[None, :] - 384 - np.arange(128)[:, None]  # [128, SLABW]
    dc = np.clip(d, 0, N - 1)
    slab_all = rel_emb[table[dc], :] * np.float32(DH)  # [128, SLABW, H] (sqrt(DH)/scale = DH)
    slab_all = np.where(d[:, :, None] >= 0, slab_all, np.float32(NEG))
    slab_all = np.ascontiguousarray(slab_all.transpose(2, 0, 1).astype(np.float32))

    g = 1.0 / (1.0 + np.exp(-gate[:, 0, 0].astype(np.float64)))
    g = g.astype(np.float32)

    mem = mem_kv.reshape(B, H, N, 2, DH)
    mk, mv = mem[..., 0, :], mem[..., 1, :]  # [B,H,N,DH]
    mmask = mem_mask.reshape(B, H, N)

    in_maps = []
    for c in range(8):
        b, h0 = c // 4, 2 * (c % 4)
        hs = [h0, h0 + 1]
        memkt = np.zeros((HPC, DH, MK), np.float32)
        memv = np.zeros((HPC, MK, DH + 1), np.float32)
        mbias = np.full((HPC, 128, MKB), NEG, np.float32)
        for i, h in enumerate(hs):
            kext = np.concatenate([null_k[None, :], mk[b, h]], axis=0)  # [N+1, DH]
            memkt[i, :, : N + 1] = kext.T
            memv[i, : N + 1, :DH] = np.concatenate([null_v[None, :], mv[b, h]], axis=0)
            memv[i, : N + 1, DH] = 1.0
            mb = np.full(MK, NEG, np.float32)
            mb[0] = 0.0
            mb[1 : N + 1] = np.where(mmask[b, h], np.float32(0.0), np.float32(NEG))
            mbias[i] = mb.reshape(MKB, 128).T
        gv = np.zeros((1, HPC * 128), np.float32)
        for i, h in enumerate(hs):
            gv[0, i * 128 : i * 128 + DH] = g[h]
            gv[0, i * 128 + DH : (i + 1) * 128] = 1.0 - g[h]
        in_maps.append(
            {
                "xb": np.ascontiguousarray(x[b]),
                "wq": np.ascontiguousarray(Wq[:, h0 * DH : (h0 + 2) * DH]),
                "wkv": np.ascontiguousarray(Wkv),
                "wout": np.ascontiguousarray(Wout[h0 * DH : (h0 + 2) * DH, :]),
                "memkt": memkt,
                "memv": memv,
                "mbias": mbias,
                "slab": np.ascontiguousarray(slab_all[h0 : h0 + 2]),
                "gvec": gv,
            }
        )
    return in_maps


_NC = None


def kernel(x, mem_kv, mem_mask, Wq, Wkv, Wout, bout, rel_emb, gate, null_k, null_v):
    global _NC
    if _NC is None:
        _NC = build_program()
    in_maps = make_in_maps(
        x, mem_kv, mem_mask, Wq, Wkv, Wout, rel_emb, gate, null_k, null_v
    )
    res = bass_utils.run_bass_kernel_spmd(_NC, in_maps, core_ids=list(range(8)))
    out = np.zeros((B, N, DIM), np.float32)
    for c in range(8):
        out[c // 4] += res.results[c]["out"]
    out += np.asarray(bout, np.float32)[None, None, :]
    return out


# revision 19
# speedup vs baseline: 1.5361x; 1.5361x over previous
"""KNN-attention Trainium2 kernel.

Sharding: 8 cores = 2 batches x 4 head-pairs. Each core computes, for its
batch b and its 2 heads: q/k/v projections, causal local attention with T5
relative bias, kNN memory attention (null key prepended), sigmoid-gated
combination, and its partial output projection (its 2 heads' rows of Wout).
Host sums the 4 partials per batch and adds bout.

Device layout: attention runs transposed (simT[j,i] = k_j . q_i) so the
softmax denominator falls out of the attn@V matmul via a ones-column on V,
and the relative-position bias + causal mask are a single Toeplitz slab
added pre-exp (logits are small enough that max-subtraction is unneeded;
masked entries use -1e33 so exp underflows to exactly 0).
"""

import sys
from contextlib import ExitStack

import numpy as np

sys.path.insert(0, "/opt/trn_rl_repo")

import concourse.bass as bass  # noqa: E402
import concourse.bacc as bacc  # noqa: E402
import concourse.mybir as mybir  # noqa: E402
import concourse.tile as tile  # noqa: E402
from concourse import bass_utils  # noqa: E402
from concourse.masks import make_identity  # noqa: E402

B, N, DIM = 2, 2048, 1024
H, DH = 8, 64
NUM_BUCKETS, MAX_DISTANCE = 32, 128
HPC = 2  # heads per core
SCALE = DH ** -0.5
MKB = 17  # mem key blocks (null + 2048 keys + pad -> 2176)
MK = MKB * 128
SLABW = 2432  # toeplitz slab width: d in [-384, 2047] -> u = d + 384
NEG = -1.0e33
F32 = mybir.dt.float32
FP = mybir.ActivationFunctionType


def _bucket_table():
    """table[d] = bucket index for key-query distance d >= 0.

    Computed with jnp so per-element numerics (log approximation, int cast)
    match the reference run in the same environment exactly — the bucket of
    boundary distances depends on the backend's log accuracy.
    """
    max_exact = NUM_BUCKETS // 2
    try:
        import jax.numpy as jnp

        n = jnp.arange(N)
        is_small = n < max_exact
        n_safe = jnp.maximum(n, 1)
        val = max_exact + (
            jnp.log(n_safe.astype(jnp.float32) / max_exact)
            / np.log(MAX_DISTANCE / max_exact)
            * (NUM_BUCKETS - max_exact)
        ).astype(jnp.int32)
        val = jnp.minimum(val, NUM_BUCKETS - 1)
        return np.asarray(jnp.where(is_small, n, val)).astype(np.int32)
    except Exception:
        d = np.arange(N)
        n_safe = np.maximum(d, 1).astype(np.float32)
        val = max_exact + (
            np.log(n_safe / np.float32(max_exact))
            / np.float32(np.log(MAX_DISTANCE / max_exact))
            * np.float32(NUM_BUCKETS - max_exact)
        ).astype(np.int32)
        val = np.minimum(val, NUM_BUCKETS - 1)
        return np.where(d < max_exact, d, val).astype(np.int32)


def build_program():
    nc = bacc.Bacc("TRN2", target_bir_lowering=False, debug=False)

    xb_d = nc.dram_tensor("xb", [N, DIM], F32, kind="ExternalInput")
    wq_d = nc.dram_tensor("wq", [DIM, HPC * DH], F32, kind="ExternalInput")
    wkv_d = nc.dram_tensor("wkv", [DIM, 2 * DH], F32, kind="ExternalInput")
    wout_d = nc.dram_tensor("wout", [HPC * DH, DIM], F32, kind="ExternalInput")
    memkt_d = nc.dram_tensor("memkt", [HPC, DH, MK], F32, kind="ExternalInput")
    memv_d = nc.dram_tensor("memv", [HPC, MK, DH + 1], F32, kind="ExternalInput")
    mbias_d = nc.dram_tensor("mbias", [HPC, 128, MKB], F32, kind="ExternalInput")
    slab_d = nc.dram_tensor("slab", [HPC, 128, SLABW], F32, kind="ExternalInput")
    gvec_d = nc.dram_tensor("gvec", [1, HPC * 128], F32, kind="ExternalInput")
    out_d = nc.dram_tensor("out", [N, DIM], F32, kind="ExternalOutput")

    with tile.TileContext(nc) as tc, ExitStack() as ctx:
        const = ctx.enter_context(tc.tile_pool(name="const", bufs=1))
        pers = ctx.enter_context(tc.tile_pool(name="pers", bufs=1))
        psum_sim = ctx.enter_context(tc.tile_pool(name="psum_sim", bufs=2, space="PSUM"))
        psum_av = ctx.enter_context(tc.tile_pool(name="psum_av", bufs=2, space="PSUM"))
        psum_misc = ctx.enter_context(tc.tile_pool(name="psum_misc", bufs=2, space="PSUM"))

        ident = const.tile([128, 128], F32)
        make_identity(nc, ident)
        zbias = const.tile([128, 1], F32)
        nc.vector.memset(zbias[:], 0.0)

        # --- persistent SBUF tensors ---
        wq_sb = pers.tile([128, 8, HPC * DH], F32)
        nc.sync.dma_start(wq_sb[:], wq_d[:].rearrange("(a p) m -> p a m", p=128))
        wkv_sb = pers.tile([128, 8, 2 * DH], F32)
        nc.sync.dma_start(wkv_sb[:], wkv_d[:].rearrange("(a p) m -> p a m", p=128))
        wout_sb = pers.tile([128, DIM], F32)
        nc.sync.dma_start(wout_sb[:], wout_d[:])
        slab_sb = pers.tile([128, HPC, SLABW], F32)
        nc.sync.dma_start(slab_sb[:], slab_d[:].rearrange("h p u -> p h u"))
        memkt_sb = pers.tile([DH, HPC, MK], F32)
        nc.sync.dma_start(memkt_sb[:], memkt_d[:].rearrange("h d k -> d h k"))
        memv_sb = pers.tile([128, HPC, MKB, DH + 1], F32)
        nc.sync.dma_start(
            memv_sb[:], memv_d[:].rearrange("h (a p) c -> p h a c", p=128)
        )
        mbias_sb = pers.tile([128, HPC, MKB], F32)
        nc.sync.dma_start(mbias_sb[:], mbias_d[:].rearrange("h p k -> p h k"))
        gvec_sb = pers.tile([DH + 1, HPC * 128], F32)  # data on partition 64
        nc.sync.dma_start(gvec_sb[DH : DH + 1, :], gvec_d[:])

        qTh = pers.tile([DH, HPC, N], F32)  # per-head qT, base partition 0
        kT = pers.tile([DH, N], F32)
        vloc = pers.tile([128, 16, DH + 1], F32)  # [key_part, key_block, DH+ones]
        nc.vector.memset(vloc[:, :, DH], 1.0)

        with (
            tc.tile_pool(name="xtpool", bufs=1) as xtpool,
            tc.tile_pool(name="xstage", bufs=3) as xstage,
            tc.tile_pool(name="qvstage", bufs=2) as qvstage,
        ):
            xT = xtpool.tile([128, 8, N], F32)  # [f_part, f_block, seq]
            vT = xtpool.tile([DH, N], F32)
            # --- transpose x into xT ---
            for t in range(16):
                xrow = xstage.tile([128, DIM], F32, tag="xrow")
                nc.sync.dma_start(xrow[:], xb_d[t * 128 : (t + 1) * 128, :])
                for fb in range(8):
                    ptr = psum_misc.tile([128, 128], F32, tag="misc")
                    nc.tensor.transpose(
                        ptr[:], xrow[:, fb * 128 : (fb + 1) * 128], ident[:]
                    )
                    nc.any.tensor_copy(
                        out=xT[:, fb, t * 128 : (t + 1) * 128], in_=ptr[:]
                    )

            # --- projections: qT (2 heads), kT/vT (shared) ---
            # Heads/v land on psum partitions 64-127; DMA re-bases them to 0.
            for icq in range(4):
                sl = slice(icq * 512, (icq + 1) * 512)
                pq = psum_misc.tile([128, 512], F32, tag="misc")
                for fb in range(8):
                    nc.tensor.matmul(
                        pq[:], wq_sb[:, fb, :], xT[:, fb, sl],
                        start=(fb == 0), stop=(fb == 7),
                    )
                nc.any.tensor_copy(out=qTh[:, 0, sl], in_=pq[:DH, :])
                qstage = qvstage.tile([128, 512], F32, tag="qstage")
                nc.any.tensor_copy(out=qstage[DH:, :], in_=pq[DH:, :])
                nc.sync.dma_start(qTh[:, 1, sl], qstage[DH:, :])
                pkv = psum_misc.tile([128, 512], F32, tag="misc")
                for fb in range(8):
                    nc.tensor.matmul(
                        pkv[:], wkv_sb[:, fb, :], xT[:, fb, sl],
                        start=(fb == 0), stop=(fb == 7),
                    )
                nc.any.tensor_copy(out=kT[:, sl], in_=pkv[:DH, :])
                vstage = qvstage.tile([128, 512], F32, tag="vstage")
                nc.any.tensor_copy(out=vstage[DH:, :], in_=pkv[DH:, :])
                nc.sync.dma_start(vT[:, sl], vstage[DH:, :])

            # vloc[p, t, :DH] = v[t*128 + p, :] via PE transpose of vT
            for t in range(16):
                ptr = psum_misc.tile([128, DH], F32, tag="misc")
                nc.tensor.transpose(
                    ptr[:], vT[:, t * 128 : (t + 1) * 128], ident[:DH, :DH]
                )
                nc.any.tensor_copy(out=vloc[:, t, :DH], in_=ptr[:])

        # --- main attention loop (pools opened after prologue pools freed) ---
        expool = ctx.enter_context(tc.tile_pool(name="expool", bufs=6))
        combpool = ctx.enter_context(tc.tile_pool(name="combpool", bufs=2))
        opool = ctx.enter_context(tc.tile_pool(name="opool", bufs=2))
        smallpool = ctx.enter_context(tc.tile_pool(name="smallpool", bufs=1))
        for ic in range(4):
            qsl = slice(ic * 512, (ic + 1) * 512)
            comb = combpool.tile([128, 512], F32, tag="comb")
            for h in range(HPC):
                hq = qTh[:, h, qsl]
                # local causal branch
                pl = psum_av.tile([DH + 1, 512], F32, tag="avL")
                nkb = 4 * (ic + 1)
                for kb in range(nkb):
                    ps = psum_sim.tile([128, 512], F32, tag="ps")
                    nc.tensor.matmul(
                        ps[:], kT[:, kb * 128 : (kb + 1) * 128], hq,
                        start=True, stop=True,
                    )
                    off = ic * 512 - kb * 128 + 384
                    nc.vector.tensor_add(ps[:], ps[:], slab_sb[:, h, off : off + 512])
                    ex = expool.tile([128, 512], F32, tag="ex")
                    nc.scalar.activation(ex[:], ps[:], FP.Exp, bias=zbias[:], scale=SCALE)
                    nc.tensor.matmul(
                        pl[:], vloc[:, kb, :], ex[:],
                        start=(kb == 0), stop=(kb == nkb - 1),
                    )
                # memory branch
                pm = psum_av.tile([DH + 1, 512], F32, tag="avM")
                for kb in range(MKB):
                    ps = psum_sim.tile([128, 512], F32, tag="ps")
                    nc.tensor.matmul(
                        ps[:], memkt_sb[:, h, kb * 128 : (kb + 1) * 128], hq,
                        start=True, stop=True,
                    )
                    ex = expool.tile([128, 512], F32, tag="ex")
                    nc.scalar.activation(
                        ex[:], ps[:], FP.Exp,
                        bias=mbias_sb[:, h, kb : kb + 1], scale=SCALE,
                    )
                    nc.tensor.matmul(
                        pm[:], memv_sb[:, h, kb, :], ex[:],
                        start=(kb == 0), stop=(kb == MKB - 1),
                    )
                # combine: comb_h = g*pl/denl + (1-g)*pm/denm
                # denominators sit on psum partition 64; stay there for the
                # reciprocal + K=1 broadcast matmuls (base 64 is legal)
                den = smallpool.tile([DH + 1, 1024], F32, tag="den")
                nc.any.tensor_copy(out=den[DH:, 0:512], in_=pl[DH:, :])
                nc.any.tensor_copy(out=den[DH:, 512:1024], in_=pm[DH:, :])
                rr = smallpool.tile([DH + 1, 1024], F32, tag="rr")
                nc.vector.reciprocal(rr[DH:, :], den[DH:, :])
                bcpL = psum_misc.tile([DH, 512], F32, tag="misc")
                nc.tensor.matmul(
                    bcpL[:], gvec_sb[DH:, h * 128 : h * 128 + DH], rr[DH:, 0:512],
                    start=True, stop=True,
                )
                bcpM = psum_misc.tile([DH, 512], F32, tag="misc")
                nc.tensor.matmul(
                    bcpM[:], gvec_sb[DH:, h * 128 + DH : (h + 1) * 128], rr[DH:, 512:1024],
                    start=True, stop=True,
                )
                bcsL = smallpool.tile([DH, 512], F32, tag="bcsL")
                nc.any.tensor_copy(out=bcsL[:], in_=bcpL[:])
                bcsM = smallpool.tile([DH, 512], F32, tag="bcsM")
                nc.any.tensor_copy(out=bcsM[:], in_=bcpM[:])
                t2 = smallpool.tile([DH, 512], F32, tag="t2")
                if h == 0:
                    ch = comb[:DH, :]
                else:
                    ch = smallpool.tile([DH, 512], F32, tag="combh1")
                nc.vector.tensor_mul(ch[:, :], pl[:DH, :], bcsL[:])
                nc.vector.tensor_mul(t2[:], pm[:DH, :], bcsM[:])
                nc.vector.tensor_add(ch[:, :], ch[:, :], t2[:])
                if h == 1:
                    nc.sync.dma_start(comb[DH:, :], ch[:, :])
            # output projection for this row chunk
            for rb in range(4):
                osb = opool.tile([128, DIM], F32, tag="osb")
                for half in range(2):
                    po = psum_misc.tile([128, 512], F32, tag="misc")
                    nc.tensor.matmul(
                        po[:], comb[:, rb * 128 : (rb + 1) * 128],
                        wout_sb[:, half * 512 : (half + 1) * 512],
                        start=True, stop=True,
                    )
                    nc.any.tensor_copy(out=osb[:, half * 512 : (half + 1) * 512], in_=po[:])
                r0 = ic * 512 + rb * 128
                nc.sync.dma_start(out_d[r0 : r0 + 128, :], osb[:])

    nc.compile()
    return nc


def make_in_maps(x, mem_kv, mem_mask, Wq, Wkv, Wout, rel_emb, gate, null_k, null_v):
    """Host-side shard + layout prep. Returns 8 per-core input dicts."""
    x = np.asarray(x, np.float32)
    mem_kv = np.asarray(mem_kv, np.float32)
    mem_mask = np.asarray(mem_mask, bool)
    Wq, Wkv, Wout = (np.asarray(a, np.float32) for a in (Wq, Wkv, Wout))
    rel_emb = np.asarray(rel_emb, np.float32)
    gate = np.asarray(gate, np.float32)
    null_k = np.asarray(null_k, np.float32)
    null_v = np.asarray(null_v, np.float32)

    table = _bucket_table()  # [N]
    # slab[h, p, u] = 64 * rel_emb[table[u-384-p], h] if d>=0 else NEG
    d = np.arange(SLABW)[None, :] - 384 - np.arange(128)[:, None]  # [128, SLABW]
    dc = np.clip(d, 0, N - 1)
    slab_all = rel_emb[table[dc], :] * np.float32(DH)  # [128, SLABW, H] (sqrt(DH)/scale = DH)
    slab_all = np.where(d[:, :, None] >= 0, slab_all, np.float32(NEG))
    slab_all = np.ascontiguousarray(slab_all.transpose(2, 0, 1).astype(np.float32))

    g = 1.0 / (1.0 + np.exp(-gate[:, 0, 0].astype(np.float64)))
    g = g.astype(np.float32)

    mem = mem_kv.reshape(B, H, N, 2, DH)
    mk, mv = mem[..., 0, :], mem[..., 1, :]  # [B,H,N,DH]
    mmask = mem_mask.reshape(B, H, N)

    in_maps = []
    for c in range(8):
        b, h0 = c // 4, 2 * (c % 4)
        hs = [h0, h0 + 1]
        memkt = np.zeros((HPC, DH, MK), np.float32)
        memv = np.zeros((HPC, MK, DH + 1), np.float32)
        mbias = np.full((HPC, 128, MKB), NEG, np.float32)
        for i, h in enumerate(hs):
            kext = np.concatenate([null_k[None, :], mk[b, h]], axis=0)  # [N+1, DH]
            memkt[i, :, : N + 1] = kext.T
            memv[i, : N + 1, :DH] = np.concatenate([null_v[None, :], mv[b, h]], axis=0)
            memv[i, : N + 1, DH] = 1.0
            mb = np.full(MK, NEG, np.float32)
            mb[0] = 0.0
            mb[1 : N + 1] = np.where(mmask[b, h], np.float32(0.0), np.float32(NEG))
            mbias[i] = mb.reshape(MKB, 128).T
        gv = np.zeros((1, HPC * 128), np.float32)
        for i, h in enumerate(hs):
            gv[0, i * 128 : i * 128 + DH] = g[h]
            gv[0, i * 128 + DH : (i + 1) * 128] = 1.0 - g[h]
        in_maps.append(
            {
                "xb": np.ascontiguousarray(x[b]),
                "wq": np.ascontiguousarray(Wq[:, h0 * DH : (h0 + 2) * DH]),
                "wkv": np.ascontiguousarray(Wkv),
                "wout": np.ascontiguousarray(Wout[h0 * DH : (h0 + 2) * DH, :]),
                "memkt": memkt,
                "memv": memv,
                "mbias": mbias,
                "slab": np.ascontiguousarray(slab_all[h0 : h0 + 2]),
                "gvec": gv,
            }
        )
    return in_maps


_NC = None


def kernel(x, mem_kv, mem_mask, Wq, Wkv, Wout, bout, rel_emb, gate, null_k, null_v):
    global _NC
    if _NC is None:
        _NC = build_program()
    in_maps = make_in_maps(
        x, mem_kv, mem_mask, Wq, Wkv, Wout, rel_emb, gate, null_k, null_v
    )
    res = bass_utils.run_bass_kernel_spmd(_NC, in_maps, core_ids=list(range(8)))
    out = np.zeros((B, N, DIM), np.float32)
    for c in range(8):
        out[c // 4] += res.results[c]["out"]
    out += np.asarray(bout, np.float32)[None, None, :]
    return out
